# revision 19
# baseline (speedup 1.0000x reference)
"""Trainium2 Bass kernel for nn_Attention (sparse_attention variant).

Reference computation (B=32, S=2048, D=512):
    energy[b,s,e] = sum_d enc[b,s,d] * W[e,d] + bias[e]
    scores[b,s]   = sum_e hidden[b,0,e] * energy[b,s,e]
    out[b,0,s]    = softmax_s(scores[b,s])

Algebraic fusion used here:
    scores[b,s] = enc[b,s,:] . v[b,:] + c[b]
      where v[b,:] = hidden[b,0,:] @ W   (tiny 32x512x512 matmul)
      and   c[b]   = hidden[b,0,:] . bias  (constant per batch -> cancels in
                                            softmax, so dropped entirely)

This turns a 34-GFLOP linear layer into a 134-MB stream of enc with one fused
multiply+reduce per tile -> the kernel is HBM-bandwidth bound.

Sharding: data-parallel over batch B across 8 NeuronCores (4 batches/core),
W replicated. No cross-device communication.
"""

import sys

if "/opt/trn_rl_repo" not in sys.path:
    sys.path.insert(0, "/opt/trn_rl_repo")

import numpy as np

import concourse.bass as bass
import concourse.bacc as bacc
import concourse.tile as tile
from concourse import bass_isa, mybir
from concourse.bass_utils import run_bass_kernel_spmd
from concourse.masks import make_identity

B, S, D = 32, 2048, 512
N_CORES = 8
B_LOC = B // N_CORES          # 4 batches per core
P = 128                       # partitions
N_SUP = 4                     # supertiles (1 MB DMA chunks) per batch
SUB = S // (N_SUP * P)        # 4 sub-tiles of 128 s-rows per supertile
N_J = S // P                  # 16 score columns per batch
EC = D // P                   # 4 contraction chunks of 128

F32 = mybir.dt.float32

_compiled = None


def _build_program():
    """Build the per-core SPMD Bass program (same program, different data)."""
    nc = bacc.Bacc("TRN2", target_bir_lowering=False, debug=False)

    enc_d = nc.dram_tensor("enc", [B_LOC, N_SUP, P, SUB, D], F32, kind="ExternalInput").ap()
    hid_d = nc.dram_tensor("hid", [B_LOC, D], F32, kind="ExternalInput").ap()
    w_d = nc.dram_tensor("w", [D, D], F32, kind="ExternalInput").ap()
    out_d = nc.dram_tensor("out", [B_LOC, N_J, P], F32, kind="ExternalOutput").ap()

    with tile.TileContext(nc) as tc:
        with (
            tc.tile_pool(name="const", bufs=1) as constp,
            tc.tile_pool(name="setup", bufs=1) as setup,
            tc.tile_pool(name="enc", bufs=8) as encp,
            tc.tile_pool(name="scratch", bufs=8) as scratchp,
            tc.tile_pool(name="soft", bufs=3) as softp,
            tc.tile_pool(name="scorep", bufs=3) as scorep,
            tc.tile_pool(name="ps_small", bufs=2, space="PSUM") as ps_small,
            tc.tile_pool(name="ps_big", bufs=2, space="PSUM") as ps_big,
        ):
            # ---- constants -------------------------------------------------
            identity = constp.tile([P, P], F32)
            make_identity(nc, identity[:, :])
            ones_row = constp.tile([1, P], F32)
            nc.gpsimd.memset(ones_row[:, :], 1.0)

            # ---- setup: v_rep[b] = broadcast(hidden[b] @ W) ----------------
            hid_sb = setup.tile([B_LOC, D], F32)
            nc.sync.dma_start(hid_sb[:, :], hid_d)
            w_sb = setup.tile([P, EC, D], F32)       # W[e,d] as [p, echunk, d]
            w_view = w_d.rearrange("(c p) d -> p c d", p=P)
            for c in range(EC):
                nc.sync.dma_start(w_sb[:, c, :], w_view[:, c, :])

            # hidden^T: [B_LOC, D] -> 4 chunks of [128e, B_LOC]
            hT = setup.tile([P, EC * B_LOC], F32)
            for c in range(EC):
                pt = ps_small.tile([P, B_LOC], F32, tag="tiny")
                nc.tensor.transpose(
                    pt[:, :],
                    hid_sb[:, c * P:(c + 1) * P],
                    identity[:B_LOC, :B_LOC],
                )
                nc.scalar.copy(hT[:, c * B_LOC:(c + 1) * B_LOC], pt[:, :])

            # PE warmup: junk transposes so the HAM clock gate sees sustained
            # activity before the latency-critical v matmuls
            for _ in range(8):
                junk = ps_big.tile([P, P], F32, tag="junk")
                nc.tensor.transpose(junk[:, :], identity[:, :], identity[:, :])

            # per-batch: v[b] = hidden[b] @ W on partition 0, then replicate
            # across all 128 partitions via ones outer-product (batch 0's
            # v_rep completes first so the DVE stream starts early)
            v_sb = setup.tile([1, B_LOC, D], F32)
            v_rep_sb = setup.tile([P, B_LOC, D], F32)
            v_rep = []
            for b in range(B_LOC):
                v_ps = ps_big.tile([1, D], F32, tag="vps")
                for c in range(EC):
                    nc.tensor.matmul(
                        v_ps[:, :],
                        hT[:, c * B_LOC + b:c * B_LOC + b + 1],
                        w_sb[:, c, :],
                        start=(c == 0),
                        stop=(c == EC - 1),
                    )
                nc.scalar.copy(v_sb[:, b, :], v_ps[:, :])
                bc = ps_big.tile([P, D], F32, tag="big")
                nc.tensor.matmul(
                    bc[:, :], ones_row[:, :], v_sb[:, b, :], start=True, stop=True
                )
                nc.scalar.copy(v_rep_sb[:, b, :], bc[:, :])
                v_rep.append(v_rep_sb[:, b, :])

            # ---- main loop: scores[b, j] = enc_tile . v[b] -----------------
            for b in range(B_LOC):
                scores = scorep.tile([P, N_J], F32, tag="scores")
                for i in range(N_SUP):
                    t = encp.tile([P, SUB, D], F32)
                    nc.sync.dma_start(t[:, :, :], enc_d[b, i])
                    for sub in range(SUB):
                        j = i * SUB + sub
                        prod = scratchp.tile([P, D], F32)
                        nc.vector.scalar_tensor_tensor(
                            out=prod[:, :],
                            in0=t[:, sub, :],
                            scalar=1.0,
                            in1=v_rep[b],
                            op0=mybir.AluOpType.mult,
                            op1=mybir.AluOpType.mult,
                            accum_out=scores[:, j:j + 1],
                        )

                # ---- softmax over the 2048 scores of batch b ---------------
                sc = scores[:, :]                         # [128, 16]
                m1 = softp.tile([P, 1], F32, tag="m1")
                nc.vector.reduce_max(m1[:, :], sc, axis=mybir.AxisListType.X)
                mall = softp.tile([P, 1], F32, tag="mall")
                nc.gpsimd.partition_all_reduce(
                    mall[:, :], m1[:, :], channels=P, reduce_op=bass_isa.ReduceOp.max
                )
                negm = softp.tile([P, 1], F32, tag="negm")
                nc.vector.tensor_scalar_mul(negm[:, :], mall[:, :], -1.0)

                probs = softp.tile([P, N_J], F32, tag="probs")
                sums = softp.tile([P, 1], F32, tag="sums")
                nc.scalar.activation(
                    probs[:, :], sc, mybir.ActivationFunctionType.Exp,
                    bias=negm[:, :], scale=1.0, accum_out=sums[:, :],
                )
                pt = ps_small.tile([N_J, P], F32, tag="tiny")
                nc.tensor.transpose(pt[:, :], probs[:, :], identity[:, :])

                sall = softp.tile([P, 1], F32, tag="sall")
                nc.gpsimd.partition_all_reduce(
                    sall[:, :], sums[:, :], channels=P, reduce_op=bass_isa.ReduceOp.add
                )
                rec = softp.tile([P, 1], F32, tag="rec")
                nc.vector.reciprocal(rec[:, :], sall[:, :])

                # normalize while copying the transposed tile out of PSUM
                # (rec holds the same value in every partition)
                ot = softp.tile([N_J, P], F32, tag="ot")
                nc.scalar.activation(
                    ot[:, :], pt[:, :], mybir.ActivationFunctionType.Copy,
                    bias=0.0, scale=rec[:N_J, :],
                )
                nc.sync.dma_start(out_d[b], ot[:, :])

    nc.compile()
    return nc


def _get_program():
    global _compiled
    if _compiled is None:
        _compiled = _build_program()
    return _compiled


def kernel(hidden, enc_outputs, W, b=None, **_unused):
    hidden = np.ascontiguousarray(np.asarray(hidden, dtype=np.float32))
    enc = np.ascontiguousarray(np.asarray(enc_outputs, dtype=np.float32))
    W = np.ascontiguousarray(np.asarray(W, dtype=np.float32))

    nc = _get_program()
    enc5 = enc.reshape(B, N_SUP, P, SUB, D)
    hid2 = hidden.reshape(B, D)
    in_maps = [
        {
            "enc": np.ascontiguousarray(enc5[c * B_LOC:(c + 1) * B_LOC]),
            "hid": np.ascontiguousarray(hid2[c * B_LOC:(c + 1) * B_LOC]),
            "w": W,
        }
        for c in range(N_CORES)
    ]
    res = run_bass_kernel_spmd(nc, in_maps, core_ids=list(range(N_CORES)))
    # device layout: out[b, j=(i, r), p] holds prob for s = i*(P*SUB) + p*SUB + r
    parts = []
    for c in range(N_CORES):
        arr = res.results[c]["out"].reshape(B_LOC, N_SUP, SUB, P)
        parts.append(arr.transpose(0, 1, 3, 2).reshape(B_LOC, 1, S))
    return np.concatenate(parts, axis=0).astype(np.float32)


if __name__ == "__main__":
    rng = np.random.default_rng(0)
    hidden = rng.standard_normal((B, 1, D), dtype=np.float32)
    enc = rng.standard_normal((B, S, D), dtype=np.float32)
    W = (rng.standard_normal((D, D), dtype=np.float32) / np.sqrt(D)).astype(np.float32)
    bias = (rng.standard_normal(D, dtype=np.float32) / np.sqrt(D)).astype(np.float32)
    out = kernel(hidden, enc, W, bias)
    v = hidden[:, 0, :] @ W
    sc = np.einsum("bsd,bd->bs", enc, v)
    e = np.exp(sc - sc.max(axis=1, keepdims=True))
    ref = (e / e.sum(axis=1, keepdims=True))[:, None, :]
    err = np.linalg.norm(out - ref) / np.linalg.norm(ref)
    print("self-check rel err:", err)


# revision 20
# speedup vs baseline: 1.0763x; 1.0763x over previous
"""Trainium2 Bass kernel for nn_Attention (sparse_attention variant).

Reference computation (B=32, S=2048, D=512):
    energy[b,s,e] = sum_d enc[b,s,d] * W[e,d] + bias[e]
    scores[b,s]   = sum_e hidden[b,0,e] * energy[b,s,e]
    out[b,0,s]    = softmax_s(scores[b,s])

Algebraic fusion used here:
    scores[b,s] = enc[b,s,:] . v[b,:] + c[b]
      where v[b,:] = hidden[b,0,:] @ W   (tiny 32x512x512 matmul)
      and   c[b]   = hidden[b,0,:] . bias  (constant per batch -> cancels in
                                            softmax, so dropped entirely)

This turns a 34-GFLOP linear layer into a 134-MB stream of enc with one fused
multiply+reduce per tile -> the kernel is HBM-bandwidth bound.

Sharding: data-parallel over batch B across 8 NeuronCores (4 batches/core),
W replicated. No cross-device communication.
"""

import sys

if "/opt/trn_rl_repo" not in sys.path:
    sys.path.insert(0, "/opt/trn_rl_repo")

import numpy as np

import concourse.bass as bass
import concourse.bacc as bacc
import concourse.tile as tile
from concourse import bass_isa, mybir
from concourse.bass_utils import run_bass_kernel_spmd
from concourse.masks import make_identity

B, S, D = 32, 2048, 512
N_CORES = 8
B_LOC = B // N_CORES          # 4 batches per core
P = 128                       # partitions
N_SUP = 4                     # supertiles (1 MB DMA chunks) per batch
SUB = S // (N_SUP * P)        # 4 sub-tiles of 128 s-rows per supertile
N_J = S // P                  # 16 score columns per batch
EC = D // P                   # 4 contraction chunks of 128

F32 = mybir.dt.float32

_compiled = None


def _build_program():
    """Build the per-core SPMD Bass program (same program, different data)."""
    nc = bacc.Bacc("TRN2", target_bir_lowering=False, debug=False)

    enc_d = nc.dram_tensor("enc", [B_LOC, N_SUP, P, SUB, D], F32, kind="ExternalInput").ap()
    hid_d = nc.dram_tensor("hid", [B_LOC, D], F32, kind="ExternalInput").ap()
    w_d = nc.dram_tensor("w", [D, D], F32, kind="ExternalInput").ap()
    out_d = nc.dram_tensor("out", [B_LOC, N_J, P], F32, kind="ExternalOutput").ap()

    with tile.TileContext(nc) as tc:
        with (
            tc.tile_pool(name="const", bufs=1) as constp,
            tc.tile_pool(name="setup", bufs=1) as setup,
            tc.tile_pool(name="enc", bufs=8) as encp,
            tc.tile_pool(name="scratch", bufs=8) as scratchp,
            tc.tile_pool(name="soft", bufs=3) as softp,
            tc.tile_pool(name="scorep", bufs=3) as scorep,
            tc.tile_pool(name="ps_small", bufs=2, space="PSUM") as ps_small,
            tc.tile_pool(name="ps_big", bufs=2, space="PSUM") as ps_big,
        ):
            # ---- constants -------------------------------------------------
            identity = constp.tile([P, P], F32)
            make_identity(nc, identity[:, :])
            ones_row = constp.tile([1, P], F32)
            nc.gpsimd.memset(ones_row[:, :], 1.0)

            # ---- setup: v_rep[b] = broadcast(hidden[b] @ W) ----------------
            hid_sb = setup.tile([B_LOC, D], F32)
            nc.sync.dma_start(hid_sb[:, :], hid_d)
            w_sb = setup.tile([P, EC, D], F32)       # W[e,d] as [p, echunk, d]
            w_view = w_d.rearrange("(c p) d -> p c d", p=P)
            for c in range(EC):
                nc.sync.dma_start(w_sb[:, c, :], w_view[:, c, :])

            # hidden^T: [B_LOC, D] -> 4 chunks of [128e, B_LOC]
            hT = setup.tile([P, EC * B_LOC], F32)
            for c in range(EC):
                pt = ps_small.tile([P, B_LOC], F32, tag="tiny")
                nc.tensor.transpose(
                    pt[:, :],
                    hid_sb[:, c * P:(c + 1) * P],
                    identity[:B_LOC, :B_LOC],
                )
                nc.scalar.copy(hT[:, c * B_LOC:(c + 1) * B_LOC], pt[:, :])

            # PE warmup: junk transposes so the HAM clock gate sees sustained
            # activity before the latency-critical v matmuls
            for _ in range(8):
                junk = ps_big.tile([P, P], F32, tag="junk")
                nc.tensor.transpose(junk[:, :], identity[:, :], identity[:, :])

            # per-batch: v[b] = hidden[b] @ W on partition 0, then replicate
            # across all 128 partitions via ones outer-product (batch 0's
            # v_rep completes first so the DVE stream starts early)
            v_sb = setup.tile([1, B_LOC, D], F32)
            v_rep_sb = setup.tile([P, B_LOC, D], F32)
            v_rep = []
            for b in range(B_LOC):
                v_ps = ps_big.tile([1, D], F32, tag="vps")
                for c in range(EC):
                    nc.tensor.matmul(
                        v_ps[:, :],
                        hT[:, c * B_LOC + b:c * B_LOC + b + 1],
                        w_sb[:, c, :],
                        start=(c == 0),
                        stop=(c == EC - 1),
                    )
                nc.scalar.copy(v_sb[:, b, :], v_ps[:, :])
                bc = ps_big.tile([P, D], F32, tag="big")
                nc.tensor.matmul(
                    bc[:, :], ones_row[:, :], v_sb[:, b, :], start=True, stop=True
                )
                nc.scalar.copy(v_rep_sb[:, b, :], bc[:, :])
                v_rep.append(v_rep_sb[:, b, :])

            # ---- main loop: scores[b, j] = enc_tile . v[b] -----------------
            for b in range(B_LOC):
                scores = scorep.tile([P, N_J], F32, tag="scores")
                for i in range(N_SUP):
                    t = encp.tile([P, SUB, D], F32)
                    nc.sync.dma_start(t[:, :, :], enc_d[b, i])
                    for sub in range(SUB):
                        j = i * SUB + sub
                        prod = scratchp.tile([P, D], F32)
                        nc.vector.scalar_tensor_tensor(
                            out=prod[:, :],
                            in0=t[:, sub, :],
                            scalar=1.0,
                            in1=v_rep[b],
                            op0=mybir.AluOpType.mult,
                            op1=mybir.AluOpType.mult,
                            accum_out=scores[:, j:j + 1],
                        )

                # ---- softmax over the 2048 scores of batch b ---------------
                sc = scores[:, :]                         # [128, 16]
                m1 = softp.tile([P, 1], F32, tag="m1")
                nc.vector.reduce_max(m1[:, :], sc, axis=mybir.AxisListType.X)
                mall = softp.tile([P, 1], F32, tag="mall")
                nc.gpsimd.partition_all_reduce(
                    mall[:, :], m1[:, :], channels=P, reduce_op=bass_isa.ReduceOp.max
                )
                negm = softp.tile([P, 1], F32, tag="negm")
                nc.vector.tensor_scalar_mul(negm[:, :], mall[:, :], -1.0)

                probs = softp.tile([P, N_J], F32, tag="probs")
                sums = softp.tile([P, 1], F32, tag="sums")
                nc.scalar.activation(
                    probs[:, :], sc, mybir.ActivationFunctionType.Exp,
                    bias=negm[:, :], scale=1.0, accum_out=sums[:, :],
                )
                pt = ps_small.tile([N_J, P], F32, tag="tiny")
                nc.tensor.transpose(pt[:, :], probs[:, :], identity[:, :])

                sall = softp.tile([P, 1], F32, tag="sall")
                nc.gpsimd.partition_all_reduce(
                    sall[:, :], sums[:, :], channels=P, reduce_op=bass_isa.ReduceOp.add
                )
                rec = softp.tile([P, 1], F32, tag="rec")
                nc.vector.reciprocal(rec[:, :], sall[:, :])

                # normalize while copying the transposed tile out of PSUM
                # (rec holds the same value in every partition)
                ot = softp.tile([N_J, P], F32, tag="ot")
                nc.scalar.activation(
                    ot[:, :], pt[:, :], mybir.ActivationFunctionType.Copy,
                    bias=0.0, scale=rec[:N_J, :],
                )
                nc.gpsimd.dma_start(out_d[b], ot[:, :])

    nc.compile()
    return nc


def _get_program():
    global _compiled
    if _compiled is None:
        _compiled = _build_program()
    return _compiled


def kernel(hidden, enc_outputs, W, b=None, **_unused):
    hidden = np.ascontiguousarray(np.asarray(hidden, dtype=np.float32))
    enc = np.ascontiguousarray(np.asarray(enc_outputs, dtype=np.float32))
    W = np.ascontiguousarray(np.asarray(W, dtype=np.float32))

    nc = _get_program()
    enc5 = enc.reshape(B, N_SUP, P, SUB, D)
    hid2 = hidden.reshape(B, D)
    in_maps = [
        {
            "enc": np.ascontiguousarray(enc5[c * B_LOC:(c + 1) * B_LOC]),
            "hid": np.ascontiguousarray(hid2[c * B_LOC:(c + 1) * B_LOC]),
            "w": W,
        }
        for c in range(N_CORES)
    ]
    res = run_bass_kernel_spmd(nc, in_maps, core_ids=list(range(N_CORES)))
    # device layout: out[b, j=(i, r), p] holds prob for s = i*(P*SUB) + p*SUB + r
    parts = []
    for c in range(N_CORES):
        arr = res.results[c]["out"].reshape(B_LOC, N_SUP, SUB, P)
        parts.append(arr.transpose(0, 1, 3, 2).reshape(B_LOC, 1, S))
    return np.concatenate(parts, axis=0).astype(np.float32)


if __name__ == "__main__":
    rng = np.random.default_rng(0)
    hidden = rng.standard_normal((B, 1, D), dtype=np.float32)
    enc = rng.standard_normal((B, S, D), dtype=np.float32)
    W = (rng.standard_normal((D, D), dtype=np.float32) / np.sqrt(D)).astype(np.float32)
    bias = (rng.standard_normal(D, dtype=np.float32) / np.sqrt(D)).astype(np.float32)
    out = kernel(hidden, enc, W, bias)
    v = hidden[:, 0, :] @ W
    sc = np.einsum("bsd,bd->bs", enc, v)
    e = np.exp(sc - sc.max(axis=1, keepdims=True))
    ref = (e / e.sum(axis=1, keepdims=True))[:, None, :]
    err = np.linalg.norm(out - ref) / np.linalg.norm(ref)
    print("self-check rel err:", err)
